# revision 88
# baseline (speedup 1.0000x reference)
"""Trainium2 Bass kernel for nn_DistributionET (nms_detection).

Computation (after dead-code elimination of the distribution head, whose
result `pointmap` is discarded by the reference):
  f2 = feats * a[col] + b[col]            (a=2,b=0 for col<30; a=1,b=1e-5 else)
  pooled = AvgPool5x5(f2); pn = relu(BN(pooled))
  co = conv1x1(pn) -> 8 ch per 8x8 window
  per-window affine M from co (rotations compose: rot(x)@rot(y) = rot(x+y);
  bottom row [0,0,1] => perspective divide is exactly 1)
  grid = M @ window_coords ; out = bilinear_sample(f2, grid)  (zeros padding,
  align_corners=True)

Device mapping (per core, 8 batches):
  - feats are uploaded as fp16 (host cast); PE transposes tiles -> [pos, ch];
    ACT/DVE apply the a/b column mask writing fp16 staging tiles
  - staging: ONE fp16 copy of the transposed masked image, row v = pixel v
    (256 ch); 1664-row per-batch pitch makes each half a rectangular DMA
  - pooling via PE matmul of the staged fp16 tiles against a constant
    pooling matrix; head + per-window trig/affine on DVE/ACT
  - bilinear gather via gpsimd dma_gather (elem_step 256 < elem_size 512):
    per position TWO int16 indices fetch rows v,v+1 (y0 corner pair) and
    v+40,v+41 (y1 pair), 1KB each; 4 instructions per batch keep the Pool
    engine's 994ns/instr SWDGE overhead off the critical path. Indices are
    wrapped into the [16, n/16] layout (replicated on all 128 partitions -
    the HW requires it) via constant shift matmuls
  - bilinear combine fused into PE "weighted transpose": out[c,p] =
    sum_k Gk^T @ diag(w_k(p)), accumulated in PSUM; diag built as an
    interleaved fp16 packed DVE op; written out as fp16, host casts back
Sharding: batch 64 = 8 cores x 8 batches (pure data parallel).
"""

import os
import numpy as np

B, C, H, W = 64, 256, 40, 40
HW = H * W
NCORES = 8
BPC = B // NCORES          # batches per core
NT = 13                    # position tiles per batch (12*128 + 64)
NW = 64                    # windows per batch
PW = (W - 1) / 2.0         # 19.5

_F32 = np.float32
_F16 = np.float16


def _np_t(t):
    return 128 if t < 12 else 64


def make_consts(bpc=BPC):
    """Pure compile-time constant tables (no input values involved)."""
    cs = {}
    p_idx = np.arange(HW)
    hh, ww = p_idx // W, p_idx % W
    win = 8 * (hh // 5) + (ww // 5)
    rr, ss = hh % 5, ww % 5

    cs["ident"] = np.eye(128, dtype=_F32)
    cs["identt"] = np.eye(128, dtype=_F16)
    # interleaved identity: idt4i[p, 4j+k] = (j==p); lets the diag-weight
    # build run fully 2-byte-packed on DVE (2x/4x mode)
    idt4i = np.zeros((128, 512), _F16)
    for p in range(128):
        idt4i[p, 4 * p:4 * p + 4] = 1.0
    cs["ident4s"] = idt4i

    # window-coord tables, pre-scaled by PW (= exactly (s-2), (r-2))
    bw = np.arange(5, dtype=np.float64) * 2.0 / (W - 1)
    bw = (bw - bw.mean()).astype(_F32)
    bh = np.arange(5, dtype=np.float64) * 2.0 / (H - 1)
    bh = (bh - bh.mean()).astype(_F32)
    wcx = np.zeros((128, NT), _F32)
    wcy = np.zeros((128, NT), _F32)
    amask = np.ones((128, NT), _F32)
    bmask = np.zeros((128, NT), _F32)
    for t in range(NT):
        for p in range(_np_t(t)):
            pp = 128 * t + p
            wcx[p, t] = np.float64(bw[ss[pp]]) * PW
            wcy[p, t] = np.float64(bh[rr[pp]]) * PW
            amask[p, t] = 2.0 if ww[pp] < 30 else 1.0
            bmask[p, t] = 0.0 if ww[pp] < 30 else 1e-5
    cs["wcx"], cs["wcy"] = wcx, wcy
    cs["amask"], cs["bmask"] = amask, bmask

    # pooling matrix [128, NT*64]: PoolA[p, 64t+w'] = 1/25 if win(p')==w'
    # (the staged f2 tiles already carry the a/b column mask)
    poola = np.zeros((128, NT * 64), np.float64)
    for t in range(NT):
        for p in range(_np_t(t)):
            pp = 128 * t + p
            poola[p, 64 * t + win[pp]] = 1.0 / 25.0
    cs["poola"] = poola.astype(_F16)

    cs["poolbm"] = np.zeros((128, 64), _F32)

    # selection matrices [NT, 64, 128]: sel[t, w, p] = 1 iff w == win(128t+p)
    selpad = np.zeros((NT, 64, 128), _F32)
    for t in range(NT):
        for p in range(_np_t(t)):
            selpad[t, win[128 * t + p], p] = 1.0
    cs["selpad"] = selpad

    # batch offsets [128, bpc*NT] col 13b+t -> 1664*b (staging row base)
    boff = np.zeros((128, bpc * NT), _F32)
    for b in range(bpc):
        boff[:, NT * b:NT * (b + 1)] = 1664.0 * b
    cs["boff"] = boff

    wrap16 = np.zeros((128, 8 * 128), _F32)
    for j in range(8):
        for i in range(128):
            wrap16[16 * j + i % 16, 128 * j + i] = 1.0
    cs["wrap16"] = wrap16

    # pack into two blobs (one DMA each): cf32 and cf16
    cs["cf32"] = np.concatenate(
        [cs["ident"], cs["wcx"], cs["wcy"], cs["amask"], cs["bmask"],
         cs["poolbm"], cs["wrap16"], cs["boff"]], axis=1)
    cs["cf16"] = np.concatenate(
        [cs["identt"], cs["ident4s"], cs["poola"]], axis=1)
    for k in ("ident", "wcx", "wcy", "amask", "bmask", "poolbm", "wrap16",
              "boff", "identt", "ident4s", "poola"):
        del cs[k]
    return cs


def build_module(bpc=BPC):
    import concourse.bass as bass
    import concourse.bacc as bacc
    import concourse.mybir as mybir
    import concourse.tile as tile
    from concourse.tile_rust import add_dep_helper
    from contextlib import ExitStack

    dt = mybir.dt
    f32 = dt.float32
    f16 = dt.float16
    A = mybir.AluOpType
    AF = mybir.ActivationFunctionType

    nc = bacc.Bacc("TRN2", target_bir_lowering=False, debug=False)

    # ---------------- DRAM tensors ----------------
    feats = nc.dram_tensor("feats", [bpc, C, HW], f16, kind="ExternalInput").ap()
    # host-packed parameter blobs (see make_in_maps)
    bnpack = nc.dram_tensor("bnpack", [128, 8], f32, kind="ExternalInput").ap()
    wdbd = nc.dram_tensor("wdbd", [8, C + 1], f32, kind="ExternalInput").ap()
    W32 = 244 + 1024 + bpc * NT
    cin = {}
    cshapes = {
        "cf32": ([128, W32], f32), "cf16": ([128, 1472], f16),
        "selpad": ([NT, 64, 128], f32),
    }
    for k, (shp, dty) in cshapes.items():
        cin[k] = nc.dram_tensor(k, shp, dty, kind="ExternalInput").ap()
    out = nc.dram_tensor("out", [bpc, C, HW], f16, kind="ExternalOutput").ap()
    # staging: single fp16 copy, row v = pixel v (256 ch). dma_gather
    # fetches contiguous row pairs (elem_step=256, elem_size=512), so the
    # 4 bilinear corners come from two indices: v (y0 pair) and v+40 (y1
    # pair). Per-batch pitch padded to 1664 = 13*128 rows so each staging
    # half is one rectangular DMA; pad rows are never gathered.
    BROW = NT * 128          # 1664
    NROW = bpc * BROW
    f2t_t = nc.dram_tensor("f2t", [NROW, C], f16)
    # overlapping windows: row v of this view = staging rows v,v+1 (512 elems)
    f2t_gather_ap = bass.AP(f2t_t, 0, [[C, NROW - 1], [1, 2 * C]])

    bufs_cfg = dict(fpool=4, f2pool=5, tpsum=4, hpsum=2, cpsum=2,
                    gpool=3, dpool=8, opool=4, wpool=2)
    bc = os.environ.get("KERNEL_BUFS", "")
    if bc:
        for kv in bc.split(","):
            k_, v_ = kv.split("=")
            bufs_cfg[k_] = int(v_)
    with tile.TileContext(nc) as tc, ExitStack() as ctx:
        cpool = ctx.enter_context(tc.tile_pool(name="consts", bufs=1))
        fpool = ctx.enter_context(tc.tile_pool(name="feats", bufs=bufs_cfg["fpool"]))
        f2pool = ctx.enter_context(tc.tile_pool(name="f2sb", bufs=bufs_cfg["f2pool"]))
        tpsum = ctx.enter_context(tc.tile_pool(name="tpsum", bufs=bufs_cfg["tpsum"], space="PSUM"))
        hpsum = ctx.enter_context(tc.tile_pool(name="hpsum", bufs=bufs_cfg["hpsum"], space="PSUM"))
        cpsum = ctx.enter_context(tc.tile_pool(name="cpsum", bufs=bufs_cfg["cpsum"], space="PSUM"))
        gpool = ctx.enter_context(tc.tile_pool(name="gath", bufs=bufs_cfg["gpool"]))
        dpool = ctx.enter_context(tc.tile_pool(name="diag", bufs=bufs_cfg["dpool"]))
        opool = ctx.enter_context(tc.tile_pool(name="outsb", bufs=bufs_cfg["opool"]))
        wpool = ctx.enter_context(
            tc.tile_pool(name="work", bufs=bufs_cfg.get("wpool", 1)))

        V = nc.vector
        S = nc.scalar
        T = nc.tensor
        DMA = nc.sync

        # ---------------- load constants (3 packed DMAs) ----------------
        cf32 = cpool.tile([128, W32], f32, tag="cf32", name="cf32")
        DMA.dma_start(cf32[:, :], cin["cf32"])
        cf16 = cpool.tile([128, 1472], f16, tag="cf16", name="cf16")
        DMA.dma_start(cf16[:, :], cin["cf16"])
        # selpad -> [64, NT*128], block t at cols 128*t
        sel_sb = cpool.tile([64, NT * 128], f32, tag="selpad")
        DMA.dma_start(
            sel_sb[:, :],
            bass.AP(cin["selpad"].tensor, 0, [[128, 64], [64 * 128, NT], [1, 128]]),
        )
        ct = {
            "ident": cf32[:, 0:128], "wcx": cf32[:, 128:141],
            "wcy": cf32[:, 141:154], "amask": cf32[:, 154:167],
            "bmask": cf32[:, 167:180], "poolbm": cf32[:, 180:244],
            "wrap16": cf32[:, 244:1268], "boff512": cf32[:, 1268:W32],
            "identt": cf16[:, 0:128], "ident4s": cf16[:, 128:640],
            "poola": cf16[:, 640:1472],
        }
        ident = ct["ident"]
        identt = ct["identt"]

        # ---------------- parameter prep (on device) ----------------
        par = wpool.tile([128, 16], f32, tag="par")
        # cols: 0:2 rv, 2:4 g, 4:6 be, 6:8 rm, 8:10 inv, 10:12 beta, 12:14 tmp
        DMA.dma_start(par[:, 0:8], bnpack)
        V.tensor_scalar(out=par[:, 12:14], in0=par[:, 0:2], scalar1=1e-5,
                        scalar2=None, op0=A.add)
        S.activation(par[:, 14:16], par[:, 12:14], AF.Sqrt)
        V.reciprocal(par[:, 12:14], par[:, 14:16])
        V.tensor_tensor(out=par[:, 8:10], in0=par[:, 12:14], in1=par[:, 2:4],
                        op=A.mult)  # inv = g / sqrt(rv+eps)
        V.tensor_tensor(out=par[:, 14:16], in0=par[:, 6:8], in1=par[:, 8:10],
                        op=A.mult)
        V.tensor_tensor(out=par[:, 10:12], in0=par[:, 4:6], in1=par[:, 14:16],
                        op=A.subtract)  # beta = be - rm*inv
        inv = par[:, 8:10]
        beta = par[:, 10:12]
        # biasw[m] = poolbm * inv_m + beta_m   [128, 64] each
        biasw = wpool.tile([128, 128], f32, tag="biasw")
        for m in range(2):
            V.scalar_tensor_tensor(
                out=biasw[:, 64 * m:64 * m + 64], in0=ct["poolbm"][:, :],
                scalar=inv[:, m:m + 1],
                in1=beta[:, m:m + 1].to_broadcast([128, 64]),
                op0=A.mult, op1=A.add)
        # wd -> wdT chunks [128, 8] x2 (PE transpose)
        wd_sb = wpool.tile([8, C + 1], f32, tag="wdsb")
        DMA.dma_start(wd_sb[:, :], wdbd)
        bd_sb = wd_sb[:, C:C + 1]
        wdT = wpool.tile([128, 16], f32, tag="wdT")
        for m in range(2):
            wtp = hpsum.tile([128, 8], f32, tag="hps")
            T.matmul(wtp[:, :], lhsT=wd_sb[:, 128 * m:128 * m + 128],
                     rhs=ident[:8, :8], is_transpose=True)
            S.activation(wdT[:, 8 * m:8 * m + 8], wtp[:, :], AF.Copy)

        # persistent coordinate tiles
        BPC_NT = bpc * NT
        x_all = wpool.tile([128, BPC_NT], f32, tag="x_all")
        y_all = wpool.tile([128, BPC_NT], f32, tag="y_all")
        vA = wpool.tile([128, BPC_NT], f32, tag="vA")
        mtiles = []
        pairtiles = {}
        pairidx = {}

        staging_writes = [[] for _ in range(bpc)]

        hold = {}

        # ================= LOOP 1: transpose/stage/pool/head =================
        fm_tiles = {}

        def load_body(b):
            fm_ = fpool.tile([128, 2 * HW], f16, tag="fm")
            DMA.dma_start(
                bass.AP(fm_.tensor, fm_[0:1, 0:1].offset,
                        [[fm_[:, :].ap[0][0], 128], [HW, 2], [1, HW]]),
                bass.AP(feats.tensor, b * C * HW,
                        [[HW, 128], [128 * HW, 2], [1, HW]]))
            fm_tiles[b] = fm_

        def loop1_body(b):
            fm_ = fm_tiles.pop(b)
            f0 = fm_[:, 0:HW]
            f1 = fm_[:, HW:2 * HW]
            pooled = [hpsum.tile([128, 64], f32, tag="hps", name=f"pool{m}")
                      for m in range(2)]
            stg = f2pool.tile([128, NT * 256], f16, tag="stg", name="stg")
            # tail tile only fills partitions 0:64; zero the rest so the
            # rectangular staging DMA reads initialized memory
            V.memset(stg[64:128, 12 * 256:NT * 256], 0.0)
            # staging: single copy, pixel v -> row (BROW*b + v); written in
            # two tile-halves so the first rectangle can go out while the
            # second half is still being masked; mask ops alternate ACT/DVE
            rows0 = BROW * b
            sstep = stg[:, :].ap[0][0]
            soff = stg[0:1, 0:1].offset

            def stage_half(th0, ntl):
                src = bass.AP(stg.tensor, soff + 256 * th0,
                              [[sstep, 128], [256, ntl], [1, 256]])
                wi = DMA.dma_start(
                    bass.AP(f2t_t, (rows0 + 128 * th0) * C,
                            [[C, 128], [128 * C, ntl], [1, C]]), src)
                staging_writes[b].append(wi)

            for t in range(NT):
                npt = _np_t(t)
                tp = tpsum.tile([128, 256], f16, tag="tp")
                for m, fap in enumerate((f0, f1)):
                    T.matmul(tp[:npt, 128 * m:128 * m + 128],
                             lhsT=fap[:, 128 * t:128 * t + npt],
                             rhs=identt[:, :], is_transpose=True)
                # a/b column mask applied while moving PSUM->SBUF (fp16)
                if t % 3 != 2:
                    S.activation(stg[:npt, 256 * t:256 * t + 256], tp[:npt, :],
                                 AF.Identity,
                                 bias=ct["bmask"][:npt, t:t + 1],
                                 scale=ct["amask"][:npt, t:t + 1])
                else:
                    V.scalar_tensor_tensor(
                        out=stg[:npt, 256 * t:256 * t + 256],
                        in0=tp[:npt, :],
                        scalar=ct["amask"][:npt, t:t + 1],
                        in1=ct["bmask"][:npt, t:t + 1].to_broadcast([npt, 256]),
                        op0=A.mult, op1=A.add)
                if t == 2 and b < 2:
                    stage_half(0, 3)
                elif t == 6:
                    stage_half(0, 7) if b >= 2 else stage_half(3, 4)
            stage_half(7, 6)
            # pooling as a second pass keeps the PE queue from ping-ponging
            # with the mask chain tile by tile
            for t in range(NT):
                npt = _np_t(t)
                for m in range(2):
                    T.matmul(pooled[m][:, :],
                             lhsT=stg[:npt, 256 * t + 128 * m:
                                      256 * t + 128 * m + 128],
                             rhs=ct["poola"][:npt, 64 * t:64 * t + 64],
                             start=(t == 0), stop=(t == NT - 1))
            # BN + relu
            pn = f2pool.tile([128, 128], f32, tag="pn")
            for m in range(2):
                V.scalar_tensor_tensor(
                    out=pn[:, 64 * m:64 * m + 64],
                    in0=pooled[m][:, :],
                    scalar=inv[:, m:m + 1],
                    in1=biasw[:, 64 * m:64 * m + 64],
                    op0=A.mult, op1=A.add)
            V.tensor_scalar(out=pn[:, :], in0=pn[:, :], scalar1=0.0,
                            scalar2=None, op0=A.max)
            # head conv -> co [8, 64]
            co = hpsum.tile([8, 64], f32, tag="hps")
            for m in range(2):
                T.matmul(co[:, :], lhsT=wdT[:, 8 * m:8 * m + 8],
                         rhs=pn[:, 64 * m:64 * m + 64],
                         start=(m == 0), stop=(m == 1))
            if b % 2 == 0:
                hold["costage"] = costage = wpool.tile(
                    [8, 128], f32, tag=f"cost{b // 2}", name=f"cost{b // 2}")
                if b == bpc - 1:  # odd bpc: zero the absent partner half
                    V.memset(costage[:, 64:128], 0.0)
            costage = hold["costage"]
            S.activation(costage[:, 64 * (b % 2):64 * (b % 2) + 64], co[:, :],
                         AF.Identity, bias=bd_sb[:, 0:1])
            if b % 2 == 1 or b == bpc - 1:
                g = b // 2
                coT = hpsum.tile([128, 8], f32, tag="hps")
                T.matmul(coT[:, :], lhsT=costage[:, :], rhs=ident[:8, :8],
                         is_transpose=True)
                mw = wpool.tile([128, 16], f32, tag=f"mw{g}")
                cow = wpool.tile([128, 8], f32, tag=f"cow{g}")
                S.activation(cow[:, :], coT[:, :], AF.Copy)
                # cow cols: 0 xr 1 yr 2 sx 3 sy 4 tx0 5 tx1 6 ty0 7 ty1
                # mw cols: 0 scos 1 ssin 2 nssin 3 TX 4 TY 5 u 6 sinu 7 cosu
                #          8 sinx 9 cosx 10 s 11..13 tmp
                c_ = lambda tl, i: tl[:, :][:, i:i + 1]
                V.tensor_tensor(out=c_(mw, 5), in0=c_(cow, 0), in1=c_(cow, 1), op=A.add)
                # trig args [u, u+pi/2, xr, xr+pi/2], range-reduced to [-pi, pi]
                targ = wpool.tile([128, 4], f32, tag="targ")
                rt0 = wpool.tile([128, 4], f32, tag="rt0")
                rt1 = wpool.tile([128, 4], f32, tag="rt1")
                rint = wpool.tile([128, 4], dt.int32, tag="rint")
                V.tensor_copy(out=targ[:, 0:1], in_=c_(mw, 5))
                V.tensor_scalar(out=targ[:, 1:2], in0=c_(mw, 5),
                                scalar1=float(np.pi / 2), scalar2=None, op0=A.add)
                V.tensor_copy(out=targ[:, 2:3], in_=c_(cow, 0))
                V.tensor_scalar(out=targ[:, 3:4], in0=c_(cow, 0),
                                scalar1=float(np.pi / 2), scalar2=None, op0=A.add)
                V.tensor_scalar(out=rt0[:, :], in0=targ[:, :],
                                scalar1=float(1.0 / (2 * np.pi)), scalar2=0.5,
                                op0=A.mult, op1=A.add)
                V.tensor_copy(out=rint[:, :], in_=rt0[:, :])
                V.tensor_copy(out=rt1[:, :], in_=rint[:, :])
                V.tensor_tensor(out=rt0[:, :], in0=rt1[:, :], in1=rt0[:, :], op=A.is_gt)
                V.tensor_tensor(out=rt1[:, :], in0=rt1[:, :], in1=rt0[:, :], op=A.subtract)
                V.scalar_tensor_tensor(out=targ[:, :], in0=rt1[:, :],
                                       scalar=float(-2 * np.pi), in1=targ[:, :],
                                       op0=A.mult, op1=A.add)
                S.activation(c_(mw, 6), targ[:, :][:, 0:1], AF.Sin)
                S.activation(c_(mw, 7), targ[:, :][:, 1:2], AF.Sin)
                S.activation(c_(mw, 8), targ[:, :][:, 2:3], AF.Sin)
                S.activation(c_(mw, 9), targ[:, :][:, 3:4], AF.Sin)
                V.tensor_tensor(out=c_(mw, 10), in0=c_(cow, 2), in1=c_(cow, 3), op=A.mult)
                V.tensor_tensor(out=c_(mw, 0), in0=c_(mw, 10), in1=c_(mw, 7), op=A.mult)
                V.tensor_tensor(out=c_(mw, 1), in0=c_(mw, 10), in1=c_(mw, 6), op=A.mult)
                V.tensor_scalar(out=c_(mw, 2), in0=c_(mw, 1), scalar1=-1.0,
                                scalar2=None, op0=A.mult)
                # TX = 19.5*((cosx*ty0 + sinx*ty1)*sx + tx0) + 19.5
                V.tensor_tensor(out=c_(mw, 11), in0=c_(mw, 9), in1=c_(cow, 6), op=A.mult)
                V.scalar_tensor_tensor(out=c_(mw, 12), in0=c_(cow, 7),
                                       scalar=c_(mw, 8), in1=c_(mw, 11),
                                       op0=A.mult, op1=A.add)
                V.scalar_tensor_tensor(out=c_(mw, 13), in0=c_(mw, 12),
                                       scalar=c_(cow, 2), in1=c_(cow, 4),
                                       op0=A.mult, op1=A.add)
                V.tensor_scalar(out=c_(mw, 3), in0=c_(mw, 13), scalar1=PW,
                                scalar2=PW, op0=A.mult, op1=A.add)
                # TY = 19.5*((-sinx*ty0 + cosx*ty1)*sx + tx1) + 19.5
                V.tensor_tensor(out=c_(mw, 11), in0=c_(mw, 9), in1=c_(cow, 7), op=A.mult)
                V.scalar_tensor_tensor(out=c_(mw, 12), in0=c_(cow, 6),
                                       scalar=c_(mw, 8), in1=c_(mw, 11),
                                       op0=A.mult, op1=A.subtract)
                V.scalar_tensor_tensor(out=c_(mw, 13), in0=c_(mw, 12),
                                       scalar=c_(cow, 2), in1=c_(cow, 5),
                                       op0=A.mult, op1=A.subtract)
                V.tensor_scalar(out=c_(mw, 4), in0=c_(mw, 13), scalar1=-PW,
                                scalar2=PW, op0=A.mult, op1=A.add)
                # shift rows 64:128 of mw down to partitions 0:64 so the
                # selection matmul (base partition 0) can read the odd batch
                mwp = hpsum.tile([64, 8], f32, tag="hps")
                T.matmul(mwp[:, 0:5], lhsT=ident[:, 64:128], rhs=mw[:, 0:5],
                         start=True, stop=True)
                mwlo = wpool.tile([64, 8], f32, tag=f"mwlo{g}")
                S.activation(mwlo[:, 0:5], mwp[:, 0:5], AF.Copy)
                mtiles.append((mw, mwlo))

        # ================= LOOP 2: per-position coords =================
        def loop2_body(b):
            mw, mwlo = mtiles[b // 2]
            rhs_sel = mw[0:64, 0:5] if b % 2 == 0 else mwlo[0:64, 0:5]
            selp = hpsum.tile([128, 65], f32, tag="hps")
            for t in range(NT):
                T.matmul(selp[:, 5 * t:5 * t + 5],
                         lhsT=sel_sb[:, 128 * t:128 * t + 128],
                         rhs=rhs_sel, start=True, stop=True)
            sl = slice(NT * b, NT * (b + 1))
            wcx_, wcy_ = ct["wcx"], ct["wcy"]
            tmp1 = wpool.tile([128, NT], f32, tag="tmp1")
            selap = selp[:, :]
            # x = wcx*scos + wcy*ssin + TX ; y = wcx*nssin + wcy*scos + TY
            for (dst, c0, c1, tc_) in ((x_all, 0, 1, 3), (y_all, 2, 0, 4)):
                V.tensor_tensor(out=tmp1[:, :], in0=wcx_[:, :],
                                in1=selap[:, c0:65:5], op=A.mult)
                V.tensor_tensor(out=dst[:, sl], in0=wcy_[:, :],
                                in1=selap[:, c1:65:5], op=A.mult)
                V.tensor_tensor(out=dst[:, sl], in0=dst[:, sl], in1=tmp1[:, :],
                                op=A.add)
                V.tensor_tensor(out=dst[:, sl], in0=dst[:, sl],
                                in1=selap[:, tc_:65:5], op=A.add)

        # ============ STAGE 7 (per batch-pair): floor/weights/indices ========
        def stage7(g):
            b0 = 2 * g
            nb = min(2, bpc - b0)
            w7 = NT * nb
            sl7 = slice(NT * b0, NT * b0 + w7)
            tmp = [wpool.tile([128, w7], f32, tag=f"t{i}", name=f"t{i}")
                   for i in range(6)]
            ti32 = wpool.tile([128, w7], dt.int32, tag="ti32", name="ti32")

            def floor_(x, xf, scratch):
                V.tensor_copy(out=ti32[:, :], in_=x)
                V.tensor_copy(out=xf[:, :], in_=ti32[:, :])
                V.tensor_tensor(out=scratch[:, :], in0=xf[:, :], in1=x, op=A.is_gt)
                V.tensor_tensor(out=xf[:, :], in0=xf[:, :], in1=scratch[:, :],
                                op=A.subtract)

            def side(xx, wa, wb, xb):
                x0, fx, e, p_, q_ = tmp[0], tmp[1], tmp[2], tmp[3], tmp[4]
                floor_(xx, x0, tmp[5])
                V.tensor_tensor(out=fx[:, :], in0=xx, in1=x0[:, :], op=A.subtract)
                V.tensor_scalar(out=xb[:, :], in0=x0[:, :], scalar1=38.0,
                                scalar2=0.0, op0=A.min, op1=A.max)
                va, vb = tmp[5], q_
                V.tensor_scalar(out=va[:, :], in0=x0[:, :], scalar1=39.0,
                                scalar2=None, op0=A.is_le)
                V.scalar_tensor_tensor(out=va[:, :], in0=x0[:, :], scalar=0.0,
                                       in1=va[:, :], op0=A.is_ge, op1=A.logical_and)
                V.tensor_scalar(out=vb[:, :], in0=x0[:, :], scalar1=38.0,
                                scalar2=None, op0=A.is_le)
                V.scalar_tensor_tensor(out=vb[:, :], in0=x0[:, :], scalar=-1.0,
                                       in1=vb[:, :], op0=A.is_ge, op1=A.logical_and)
                V.tensor_tensor(out=e[:, :], in0=x0[:, :], in1=xb[:, :], op=A.is_equal)
                V.tensor_scalar(out=p_[:, :], in0=fx[:, :], scalar1=-1.0,
                                scalar2=1.0, op0=A.mult, op1=A.add)
                V.tensor_tensor(out=p_[:, :], in0=p_[:, :], in1=va[:, :], op=A.mult)
                V.tensor_tensor(out=q_[:, :], in0=fx[:, :], in1=vb[:, :], op=A.mult)
                V.tensor_tensor(out=tmp[1][:, :], in0=p_[:, :], in1=q_[:, :],
                                op=A.subtract)
                V.tensor_tensor(out=wa[:, :], in0=e[:, :], in1=tmp[1][:, :], op=A.mult)
                V.tensor_tensor(out=wa[:, :], in0=wa[:, :], in1=q_[:, :], op=A.add)
                V.tensor_tensor(out=wb[:, :], in0=p_[:, :], in1=q_[:, :], op=A.add)
                V.tensor_tensor(out=wb[:, :], in0=wb[:, :], in1=wa[:, :], op=A.subtract)

            wax = wpool.tile([128, w7], f32, tag="wax", name="wax")
            wbx = wpool.tile([128, w7], f32, tag="wbx", name="wbx")
            xbt = wpool.tile([128, w7], f32, tag="xbt", name="xbt")
            uay = wpool.tile([128, w7], f32, tag="uay", name="uay")
            uby = wpool.tile([128, w7], f32, tag="uby", name="uby")
            ybt = wpool.tile([128, w7], f32, tag="ybt", name="ybt")
            side(x_all[:, sl7], wax, wbx, xbt)
            side(y_all[:, sl7], uay, uby, ybt)
            # wquad col 4*col+k = weight k of position-tile col; block order
            # matches the gathered rows [pix v, pix v+40, pix v+1, pix v+41]:
            # k: 0=(y0,x0) 1=(y1,x0) 2=(y0,x1) 3=(y1,x1)
            # idx32/wquad rotate per pair so gathers of the previous pair
            # don't serialize against this pair's writes
            wquad = wpool.tile([128, 4 * w7], f16, tag=f"wq{g % 3}",
                               name=f"wq{g % 3}")
            idx32 = wpool.tile([128, w7], dt.int16, tag=f"ix{g % 3}",
                               name=f"ix{g % 3}")
            pairtiles[g] = (idx32, wquad)
            pstep = wquad[:, :].ap[0][0]
            for k, (uu, ww_) in enumerate(((uay, wax), (uby, wax),
                                           (uay, wbx), (uby, wbx))):
                V.tensor_tensor(
                    out=bass.AP(wquad.tensor, wquad[0:1, 0:1].offset + k,
                                [[pstep, 128], [4, w7]]),
                    in0=uu[:, :], in1=ww_[:, :], op=A.mult)
            V.scalar_tensor_tensor(out=vA[:, sl7], in0=ybt[:, :], scalar=40.0,
                                   in1=xbt[:, :], op0=A.mult, op1=A.add)
            V.tensor_tensor(out=vA[:, sl7], in0=vA[:, sl7],
                            in1=ct["boff512"][:, sl7], op=A.add)
            # wrap indices into dma_gather's [16, n/16] layout:
            # idx16[q, 104h + 8t + j] = vA[16j+q, NT*h + t] via shift
            # matmuls (partition moves need PE) + strided int16 converts,
            # one pass per pair covering both batches. Two tiles per pair:
            # Y0 rows (v) and Y1 rows (v+40)
            idx16a = wpool.tile([128, 8 * NT * 2], dt.int16,
                                tag=f"i16a_{g % 3}", name=f"i16a_{g % 3}")
            idx16b = wpool.tile([128, 8 * NT * 2], dt.int16,
                                tag=f"i16b_{g % 3}", name=f"i16b_{g % 3}")
            astep = idx16a[:, :].ap[0][0]
            aoff = idx16a[0:1, 0:1].offset
            bstep = idx16b[:, :].ap[0][0]
            boff_ = idx16b[0:1, 0:1].offset
            for j in range(8):
                shp = hpsum.tile([128, 2 * NT], f32, tag="hps")
                T.matmul(shp[:, 0:w7],
                         lhsT=ct["wrap16"][:, 128 * j:128 * j + 128],
                         rhs=vA[:, sl7], start=True, stop=True)
                sstep2 = shp[:, :].ap[0][0]
                soff2 = shp[0:1, 0:1].offset
                shp_v = bass.AP(shp.tensor, soff2,
                                [[sstep2, 128], [NT, nb], [1, NT]])
                V.tensor_copy(
                    out=bass.AP(idx16a.tensor, aoff + j,
                                [[astep, 128], [8 * NT, nb], [8, NT]]),
                    in_=shp_v)
                V.tensor_scalar(
                    out=bass.AP(idx16b.tensor, boff_ + j,
                                [[bstep, 128], [8 * NT, nb], [8, NT]]),
                    in0=shp_v, scalar1=40.0, scalar2=None, op0=A.add)
            pairidx[g] = (idx16a, idx16b)

        # ================= LOOP 3: gather + combine + out =================
        GSPLIT = ((0, 7, 896), (7, 6, 704))   # (t0, nslots, num_idxs)

        def loop3_body(b):
            osb = opool.tile([128, 3200], f16, tag="osb", name="osb")
            idx32, wquad = pairtiles[b // 2]
            idx16a, idx16b = pairidx[b // 2]
            icol = 8 * NT * (b % 2)
            # pre-build all diag-weight tiles (DVE) so the PE combine chain
            # never waits on DVE tile by tile
            dts = []
            for t in range(NT):
                col = NT * (b % 2) + t
                d_all = dpool.tile([128, 512], f16, tag="d_all",
                                   name="d_all")
                # d_all[p, 4j+k] = (j==p) * w_k(p): fully packed fp16 op
                wq_ap = bass.AP(wquad.tensor, wquad[0:1, 0:1].offset + 4 * col,
                                [[wquad[:, :].ap[0][0], 128],
                                 [0, 128], [1, 4]])
                V.tensor_tensor(out=d_all[:, :], in0=ct["ident4s"][:, :],
                                in1=wq_ap, op=A.mult)
                dts.append(d_all)
            gtiles = {}
            for (t0, nsl, nidx) in GSPLIT:
                pair_g = []
                for y, idx16 in enumerate((idx16a, idx16b)):
                    g_ = gpool.tile([128, nsl * 512], f16, tag=f"g{t0}{y}",
                                    name=f"g{t0}{y}")
                    gstep = g_[:, :].ap[0][0]
                    goff = g_[0:1, 0:1].offset
                    gi = nc.gpsimd.dma_gather(
                        out_ap=bass.AP(g_.tensor, goff,
                                       [[gstep, 128], [512, nsl], [1, 512]]),
                        in_ap=f2t_gather_ap,
                        idxs_ap=idx16[:, icol + 8 * t0:
                                      icol + 8 * t0 + (nidx + 15) // 16],
                        num_idxs=nidx, num_idxs_reg=nidx,
                        elem_size=512, elem_step=C)
                    for wi_ in staging_writes[b]:
                        add_dep_helper(gi.ins, wi_.ins,
                                       reason="staging->gather")
                    pair_g.append(g_)
                gtiles[t0] = pair_g
            for t in range(NT):
                npt = _np_t(t)
                col = NT * (b % 2) + t
                for (t0, nsl, nidx) in GSPLIT:
                    if t0 <= t < t0 + nsl:
                        gpair, tl = gtiles[t0], t - t0
                        break
                cps = cpsum.tile([128, 256], f32, tag="cp", name="cp")
                d_all = dts[t]
                dstep = d_all[:, :].ap[0][0]
                doff = d_all[0:1, 0:1].offset
                # k order: 0=(y0,x0) 1=(y1,x0) 2=(y0,x1) 3=(y1,x1);
                # gather tile = y-side (k%2), block half = x-side (k//2)
                for m in range(2):
                    for k in range(4):
                        gy = gpair[k % 2]
                        T.matmul(cps[:, 128 * m:128 * m + npt],
                                 lhsT=gy[:npt, 512 * tl + 256 * (k // 2)
                                         + 128 * m:
                                         512 * tl + 256 * (k // 2)
                                         + 128 * m + 128],
                                 rhs=bass.AP(d_all.tensor, doff + k,
                                             [[dstep, npt], [4, npt]]),
                                 start=(k == 0), stop=(k == 3))
                # drain both halves in one op, alternating ACT/DVE
                ops = osb[:, :].ap[0][0]
                dst = bass.AP(osb.tensor, osb[0:1, 0:1].offset + 128 * t,
                              [[ops, 128], [1600, 2], [1, npt]])
                cstep = cps[:, :].ap[0][0]
                src = bass.AP(cps.tensor, cps[0:1, 0:1].offset,
                              [[cstep, 128], [128, 2], [1, npt]])
                S.activation(dst, src, AF.Copy)
            # two half-writes let the first dispatch while later tiles drain
            for (c0, cw) in ((0, 256), (256, 256), (512, 256), (768, 256), (1024, 256), (1280, 320)):
                DMA.dma_start(
                    bass.AP(out.tensor, b * C * HW + c0,
                            [[HW, 128], [128 * HW, 2], [1, cw]]),
                    bass.AP(osb.tensor, osb[0:1, 0:1].offset + c0,
                            [[osb[:, :].ap[0][0], 128], [1600, 2], [1, cw]]))

        # ================= driver: per-pair pipeline =================
        npair = (bpc + 1) // 2
        depth = int(os.environ.get("KERNEL_SWPIPE", "3"))
        if depth > 0:
            split3 = os.environ.get("KERNEL_SPLIT3", "0") == "1"
            for b in range(min(2, bpc)):
                load_body(b)
            for g in range(npair):
                pb = range(2 * g, min(2 * g + 2, bpc))
                for b in range(2 * g + 2, min(2 * g + 4, bpc)):
                    load_body(b)
                for b in pb:
                    loop1_body(b)
                lag = list(range(2 * (g - depth),
                                 min(2 * (g - depth) + 2, bpc))) if g >= depth else []
                if split3 and lag:
                    loop3_body(lag[0])
                for b in pb:
                    loop2_body(b)
                stage7(g)
                for b in (lag[1:] if split3 else lag):
                    loop3_body(b)
            for g in range(max(0, npair - depth), npair):
                for b in range(2 * g, min(2 * g + 2, bpc)):
                    loop3_body(b)
        else:
            for g in range(npair):
                pb = range(2 * g, min(2 * g + 2, bpc))
                for b in pb:
                    load_body(b)
                    loop1_body(b)
                for b in pb:
                    loop2_body(b)
                stage7(g)
                for b in pb:
                    loop3_body(b)
        del depth

    nc.compile()
    return nc


_MODULE_CACHE = {}


def _get_module(bpc=BPC):
    key = (bpc,)
    if key not in _MODULE_CACHE:
        _MODULE_CACHE[key] = build_module(bpc)
    return _MODULE_CACHE[key]


def make_in_maps(inputs, bpc=BPC, n_cores=NCORES):
    cs = make_consts(bpc)
    feats = np.asarray(inputs["feats"], np.float16).reshape(B, C, HW)
    base = {k: np.ascontiguousarray(v) for k, v in cs.items()}
    # host-packed params: bnpack cols 0:2 rv, 2:4 g, 4:6 be, 6:8 rm,
    # each [C]->[128, 2] column-major pair; wdbd = [wd | bd]
    bn = np.empty((128, 8), np.float32)
    for j, k in enumerate(("rvd", "gd", "bed", "rmd")):
        v = np.asarray(inputs[k], np.float32)
        bn[:, 2 * j] = v[0:128]
        bn[:, 2 * j + 1] = v[128:256]
    base["bnpack"] = bn
    wdbd = np.concatenate(
        [np.asarray(inputs["wd"], np.float32),
         np.asarray(inputs["bd"], np.float32).reshape(8, 1)], axis=1)
    base["wdbd"] = np.ascontiguousarray(wdbd)
    in_maps = []
    for i in range(n_cores):
        m = dict(base)
        m["feats"] = np.ascontiguousarray(feats[bpc * i:bpc * (i + 1)])
        in_maps.append(m)
    return in_maps


def kernel(**inputs) -> np.ndarray:
    from concourse.bass_utils import run_bass_kernel_spmd
    nc = _get_module()
    in_maps = make_in_maps(inputs)
    trace = bool(int(os.environ.get("KERNEL_TRACE", "0")))
    res = run_bass_kernel_spmd(nc, in_maps, list(range(NCORES)), trace=trace)
    if trace and res.exec_time_ns is not None:
        print(f"HW exec time: {res.exec_time_ns} ns")
    out = np.concatenate([r["out"] for r in res.results], axis=0)
    return np.ascontiguousarray(out.reshape(B, C, H, W).astype(np.float32))


if __name__ == "__main__":
    nc = build_module()
    print("module built ok")
